# revision 37
# baseline (speedup 1.0000x reference)
"""Trainium2 Bass kernel for nn_DepartmentClassifierRNN.

2-layer tanh RNN, V=32000, E=H=512, O=32, B=64, T=512.

Sharding: data-parallel over batch across 8 NeuronCores (8 examples per
core); small weights replicated, the sequential time loop stays local per
device.

Per-core structure (fp16 data, fp8-e3m4 recurrent weights, fp32 PSUM):
  * Host precomputes F0 = (emb @ Whx0.T + b_h0) * 64 (a token-independent,
    weight-only transform) so layer-1 input projections become table rows.
  * The device gathers this core's F0 rows by token id with indirect DMA
    (4 timesteps per [128, 128] tile, at partition bases 0/32/64/96).
  * Layer-1 recurrence: per step, one "selection" matmul (lhsT = gathered
    rows) drops the input projections into PSUM — the PE transposes the
    rows for free — then 16 Whh0 matmuls accumulate h-to-h; ScalarE
    applies tanh(z/64) into h1_all. Whh0/Whh1 are stored as fp8 e3m4
    scaled by 64 (rel err ~8.5e-3 vs the 2e-2 gate).
  * pre2 = Whx1 @ h1 + b_h1 (x64) as batched matmuls per 16-step block;
    the PSUM->SBUF bias-copy runs on VectorE, keeping ScalarE free — the
    per-chain serial cycle (tanh round trip + dependent matmul burst,
    ~800ns) is the wall, and ScalarE sits on it.
  * Layer-2 recurrence mirrors layer 1 (pre2 injected via an identity
    matmul); VectorE captures h2 at t = seq_len-1 per example with a
    predicated copy against a precomputed mask.
  * Final projections y = Wyh1 @ h2_sel + b_y1, out = Wf @ y + bf.
  * The two layer chains are software-pipelined with a short LAG (24
    steps — the tail costs LAG x ~800ns) so each chain's tanh/sync
    latency hides under the other chain's TensorE work.
"""

import sys

sys.path.insert(0, "/opt/trn_rl_repo")

import numpy as np
import concourse.bass as bass
import concourse.mybir as mybir
from concourse import tile
from concourse.bass_utils import run_bass_kernel_spmd

FP16 = mybir.dt.float16
FP8 = mybir.dt.float8e3  # e3m4: 4 mantissa bits
FP32 = mybir.dt.float32
I32 = mybir.dt.int32
WS = 64.0  # fp8 weight scale: Whh*WS lands in e3m4's normal range

V, E, H, O, L = 32000, 512, 512, 32, 2
B, T = 64, 512
NCORES = 8
BL = B // NCORES  # 8 examples per core
KC = H // 128  # contraction chunks
MC = H // 128  # output chunks


def _split_excess_waits(nc, max_waits=1):
    """The walrus build in this container rejects >1 sem-wait per
    instruction; spill extra waits onto preceding NoOps (same engine)."""
    for fn in nc.m.functions:
        for b in fn.blocks:
            new_insts = []
            for inst in b.instructions:
                si = inst.sync_info
                if si is not None and si.on_wait and len(si.on_wait) > max_waits:
                    waits = list(si.on_wait)
                    overflow, keep = waits[:-max_waits], waits[-max_waits:]
                    for i in range(0, len(overflow), max_waits):
                        chunk = overflow[i : i + max_waits]
                        nop = mybir.InstNoOp(
                            name=nc.get_next_instruction_name(), ins=[], outs=[]
                        )
                        nop.engine = inst.engine
                        nop.sync_info = mybir.SyncInfo(on_wait=chunk, on_update=[])
                        nc.register_instruction(nop)
                        new_insts.append(nop)
                    si.on_wait = keep
                new_insts.append(inst)
            b.instructions = new_insts


def build_nc(
    T=T, gather_bufs=16, zbufs=4, interleave=True, reps=1, TB=16, lag=None, probe_mcut=None
):
    nc = bass.Bass()
    NGT = T // 4

    # F0 viewed as quarter-rows: row v*4+m = chunk m (128 elems) of F0[v]
    f0_d = nc.dram_tensor("f0", [V * 4, 128], FP16, kind="ExternalInput")
    gidx_d = nc.dram_tensor("gidx", [128, NGT], I32, kind="ExternalInput")
    whh0_d = nc.dram_tensor("whh0t", [128, KC * H], FP8, kind="ExternalInput")
    whh1_d = nc.dram_tensor("whh1t", [128, KC * H], FP8, kind="ExternalInput")
    whx1_d = nc.dram_tensor("whx1t", [128, KC * H], FP16, kind="ExternalInput")
    wyh1_d = nc.dram_tensor("wyh1t", [128, KC * H], FP16, kind="ExternalInput")
    wft_d = nc.dram_tensor("wft", [128, KC * O], FP16, kind="ExternalInput")
    s128_d = nc.dram_tensor("s128", [128, 32], FP16, kind="ExternalInput")
    i128_d = nc.dram_tensor("i128", [128, 128], FP16, kind="ExternalInput")
    bh1_d = nc.dram_tensor("bh1", [128, MC], FP32, kind="ExternalInput")
    by1_d = nc.dram_tensor("by1", [128, MC], FP32, kind="ExternalInput")
    bfb_d = nc.dram_tensor("bfb", [O, 1], FP32, kind="ExternalInput")
    mask_d = nc.dram_tensor(
        "selmask", [128, T * 32], mybir.dt.int8, kind="ExternalInput"
    )
    out_d = nc.dram_tensor("out", [O, BL], FP32, kind="ExternalOutput")

    Tanh = mybir.ActivationFunctionType.Tanh
    Ident = mybir.ActivationFunctionType.Identity

    with tile.TileContext(nc) as tc:
        with (
            tc.tile_pool(name="const", bufs=1) as cpool,
            tc.tile_pool(name="state", bufs=1) as spool,
            tc.tile_pool(name="gath", bufs=gather_bufs) as gpool,
            tc.tile_pool(name="zps", bufs=zbufs, space="PSUM") as zpool,
            tc.tile_pool(name="p2ps", bufs=2, space="PSUM") as p2pool,
        ):
            def load(dram, shape, dtype):
                t = cpool.tile(shape, dtype, tag=dram.name)
                nc.sync.dma_start(t[:], dram.ap())
                return t

            whh0 = load(whh0_d, [128, KC * H], FP8)
            whh1 = load(whh1_d, [128, KC * H], FP8)
            whx1 = load(whx1_d, [128, KC * H], FP16)
            wyh1 = load(wyh1_d, [128, KC * H], FP16)
            wft = load(wft_d, [128, KC * O], FP16)
            s128 = load(s128_d, [128, 32], FP16)
            i128 = load(i128_d, [128, 128], FP16)
            bh1 = load(bh1_d, [128, MC], FP32)
            by1 = load(by1_d, [128, MC], FP32)
            bfb = load(bfb_d, [O, 1], FP32)
            gidx = load(gidx_d, [128, NGT], I32)
            mask = load(mask_d, [128, T * 32], mybir.dt.int8)

            hinit = cpool.tile([128, 32], FP16, tag="hinit")
            nc.gpsimd.memset(hinit[:], 0.0)
            zmask = cpool.tile([128, 32], mybir.dt.int8, tag="zmask")
            nc.gpsimd.memset(zmask[:], 0)

            TB = min(TB, T)
            # pre2 block nt is emitted at it = TB*(nt+1) .. TB*(nt+1)+MC-1
            # (one group per iteration), so layer-2 step s=nt*TB at
            # it=s+LAG is safe for LAG >= TB + MC
            LAG = lag if lag is not None else TB + MC
            NIT = T + LAG
            h1_all = spool.tile([128, T * 32], FP16, tag="h1_all")
            h2_all = spool.tile([128, T * 32], FP16, tag="h2_all")
            pre2 = spool.tile([128, T * 32], FP16, tag="pre2")
            h2sel = spool.tile([128, 32], FP16, tag="h2sel")
            nc.gpsimd.memset(h2sel[:], 0.0)

            h1v = h1_all[:].rearrange("p (t k b) -> p t k b", k=KC, b=8)
            p2v = pre2[:].rearrange("p (t m b) -> p t m b", m=MC, b=8)
            gtiles = {}

            def emit_gather(g):
                gt = gpool.tile([128, 128], FP16, tag="gt")
                nc.gpsimd.indirect_dma_start(
                    out=gt[:],
                    out_offset=None,
                    in_=f0_d.ap(),
                    in_offset=bass.IndirectOffsetOnAxis(
                        ap=gidx[:, g : g + 1], axis=0
                    ),
                )
                gtiles[g] = gt

            def emit_p1_step(t, mcut=probe_mcut or MC):
                g, q = t // 4, t % 4
                if q == 0 and g not in gtiles:
                    emit_gather(g)
                gt = gtiles[g]
                z = zpool.tile([128, 32], FP32, tag="z")
                # one matmul injects the whole step's input projections:
                # out[p, m*8+b] = gt[q*32+m*8+b, p] = F0[x[b,t]][m*128+p].
                # It's chain-independent, giving the PE runway while the
                # previous tanh drains; start=True sets all has_written
                # bits so the Whh matmuls below accumulate.
                nc.tensor.matmul(
                    z[:],
                    lhsT=gt[:],
                    rhs=i128[:, q * 32 : (q + 1) * 32],
                    start=True,
                    stop=False,
                    skip_group_check=True,
                )
                for m in range(mcut):
                    zs = z[:, m * 8 : (m + 1) * 8]
                    for k in range(KC):
                        rhs = (
                            h1_all[:, (t - 1) * 32 + k * 8 : (t - 1) * 32 + k * 8 + 8]
                            if t > 0
                            else hinit[:, k * 8 : k * 8 + 8]
                        )
                        nc.tensor.matmul(
                            zs,
                            lhsT=whh0[:, k * H + m * 128 : k * H + (m + 1) * 128],
                            rhs=rhs,
                            start=False,
                            stop=(m == mcut - 1 and k == KC - 1),
                            skip_group_check=True,
                        )
                nc.scalar.activation(
                    h1_all[:, t * 32 : (t + 1) * 32], z[:], Tanh, scale=1.0 / WS
                )

            def emit_pre2_group(nt, m):
                ts = slice(nt * TB, (nt + 1) * TB)
                pz = p2pool.tile([128, TB * 8], FP32, tag="pz")
                for k in range(KC):
                    nc.tensor.matmul(
                        pz[:],
                        lhsT=whx1[:, k * H + m * 128 : k * H + (m + 1) * 128],
                        rhs=h1v[:, ts, k, :],
                        start=(k == 0),
                        stop=(k == KC - 1),
                    )
                # bias-add + PSUM->SBUF on the (lightly loaded) vector
                # engine, keeping ScalarE free for the per-step tanhs
                nc.vector.tensor_scalar_add(
                    p2v[:, ts, m, :],
                    pz[:].rearrange("p (t b) -> p t b", b=8),
                    bh1[:, m : m + 1],
                )

            def emit_p2_step(s, mcut=probe_mcut or MC):
                z = zpool.tile([128, 32], FP32, tag="z")
                # one identity matmul injects the whole step's pre2 into the
                # bank: out[p, c] = pre2[p, s*32+c] (layout matches exactly)
                nc.tensor.matmul(
                    z[:],
                    lhsT=i128[:],
                    rhs=pre2[:, s * 32 : (s + 1) * 32],
                    start=True,
                    stop=False,
                    skip_group_check=True,
                )
                for m in range(mcut):
                    zs = z[:, m * 8 : (m + 1) * 8]
                    for k in range(KC):
                        rhs = (
                            h2_all[:, (s - 1) * 32 + k * 8 : (s - 1) * 32 + k * 8 + 8]
                            if s > 0
                            else hinit[:, k * 8 : k * 8 + 8]
                        )
                        nc.tensor.matmul(
                            zs,
                            lhsT=whh1[:, k * H + m * 128 : k * H + (m + 1) * 128],
                            rhs=rhs,
                            start=False,
                            stop=(m == mcut - 1 and k == KC - 1),
                            skip_group_check=True,
                        )
                nc.scalar.activation(
                    h2_all[:, s * 32 : (s + 1) * 32], z[:], Tanh, scale=1.0 / WS
                )
                nc.vector.copy_predicated(
                    h2sel[:],
                    mask[:, s * 32 : (s + 1) * 32],
                    h2_all[:, s * 32 : (s + 1) * 32],
                )

            for rep in range(reps):
                gtiles.clear()
                if rep > 0:
                    # no-op write that makes this rep's t=0 (which reads
                    # hinit) depend on the previous rep's final h2 — forces
                    # strict rep serialization so reps-differencing measures
                    # true single-shot latency.
                    nc.vector.copy_predicated(
                        hinit[:], zmask[:], h2_all[:, (T - 1) * 32 : T * 32]
                    )
                pre2_queue = []
                for it in range(NIT):
                    if it < T:
                        emit_p1_step(it)
                    if it % TB == 0 and 1 <= it // TB <= T // TB:
                        nt = it // TB - 1
                        pre2_queue.extend((nt, m) for m in range(MC))
                    if pre2_queue:
                        emit_pre2_group(*pre2_queue.pop(0))
                    s = it - LAG
                    if s >= 0:
                        emit_p2_step(s)

            # ---- y = Wyh1 @ h2sel + b_y1 ; out = Wf @ y + bf ----
            y_sb = spool.tile([128, 32], FP16, tag="y_sb")
            with tc.tile_pool(name="fps", bufs=1, space="PSUM") as fpool:
                yz = fpool.tile([128, 32], FP32, tag="yz")
                for m in range(MC):
                    for k in range(KC):
                        nc.tensor.matmul(
                            yz[:, m * 8 : (m + 1) * 8],
                            lhsT=wyh1[:, k * H + m * 128 : k * H + (m + 1) * 128],
                            rhs=h2sel[:, k * 8 : (k + 1) * 8],
                            start=(k == 0),
                            stop=(k == KC - 1),
                        )
                for m in range(MC):
                    nc.scalar.activation(
                        y_sb[:, m * 8 : (m + 1) * 8],
                        yz[:, m * 8 : (m + 1) * 8],
                        Ident,
                        bias=by1[:, m : m + 1],
                    )
                fz = fpool.tile([O, 8], FP32, tag="fz")
                for k in range(KC):
                    nc.tensor.matmul(
                        fz[:],
                        lhsT=wft[:, k * O : (k + 1) * O],
                        rhs=y_sb[:, k * 8 : (k + 1) * 8],
                        start=(k == 0),
                        stop=(k == KC - 1),
                    )
                out_sb = spool.tile([O, 8], FP32, tag="out_sb")
                nc.scalar.activation(out_sb[:], fz[:], Ident, bias=bfb[:, 0:1])
                nc.sync.dma_start(out_d.ap(), out_sb[:])

    _split_excess_waits(nc, max_waits=1)
    return nc


# ---------------- host-side preparation ----------------


def _tile_w(w, scale=1.0, dtype=np.float16):
    """[out,in] weight -> stationary-operand layout [128, kc*Hout + m]."""
    wt = w.T.astype(np.float32) * scale
    return (
        wt.reshape(KC, 128, w.shape[0]).transpose(1, 0, 2).reshape(128, -1)
    ).astype(dtype)


def _prep_shared(inputs):
    emb = np.asarray(inputs["emb"], np.float32)
    Whx = np.asarray(inputs["Whx"], np.float32)
    Whh = np.asarray(inputs["Whh"], np.float32)
    b_h = np.asarray(inputs["b_h"], np.float32)
    Wyh = np.asarray(inputs["Wyh"], np.float32)
    b_y = np.asarray(inputs["b_y"], np.float32)
    Wf = np.asarray(inputs["Wf"], np.float32)
    bf = np.asarray(inputs["bf"], np.float32)

    import ml_dtypes

    f8 = ml_dtypes.float8_e3m4
    f0 = ((emb @ Whx[0].T + b_h[0]) * WS).astype(np.float16).reshape(V * 4, 128)
    s128 = np.zeros((128, 32), np.float16)
    for g in range(4):
        for b in range(8):
            s128[g * 32 + b, g * 8 + b] = 1.0
    return {
        "f0": f0,
        "whh0t": _tile_w(Whh[0], WS, f8),
        "whh1t": _tile_w(Whh[1], WS, f8),
        "whx1t": _tile_w(Whx[1], WS),
        "wyh1t": _tile_w(Wyh[1]),
        "wft": _tile_w(Wf),
        "s128": s128,
        "i128": np.eye(128, dtype=np.float16),
        "bh1": np.ascontiguousarray(
            b_h[1].reshape(MC, 128).T * WS, dtype=np.float32
        ),
        "by1": np.ascontiguousarray(b_y[1].reshape(MC, 128).T, dtype=np.float32),
        "bfb": bf.reshape(O, 1).astype(np.float32),
    }


def _prep_core(inputs, core, Tk=T):
    x = np.asarray(inputs["x"]).astype(np.int64).astype(np.int32)
    sl = np.asarray(inputs["sequence_lengths"]).astype(np.int64).astype(np.int32)
    xc = x[core * BL : (core + 1) * BL]
    slc = sl[core * BL : (core + 1) * BL]
    NGT = Tk // 4
    gidx = np.zeros((128, NGT), np.int32)
    for p in range(128):
        q, r = p // 32, p % 32
        m, b = r // 8, r % 8
        gidx[p, :] = xc[b, q::4][:NGT] * 4 + m
    mask = np.zeros((128, Tk, MC, BL), np.int8)
    for b in range(BL):
        mask[:, slc[b] - 1, :, b] = 1
    return {"gidx": gidx, "selmask": mask.reshape(128, Tk * 32)}


def make_in_maps(inputs, Tk=T):
    shared = _prep_shared(inputs)
    return [dict(shared, **_prep_core(inputs, c, Tk)) for c in range(NCORES)]


def assemble_out(results):
    out = np.zeros((B, O), np.float32)
    for c in range(NCORES):
        out[c * BL : (c + 1) * BL, :] = results[c]["out"].T
    return out


_NC_CACHE = {}


def kernel(**inputs) -> np.ndarray:
    if "nc" not in _NC_CACHE:
        _NC_CACHE["nc"] = build_nc()
    nc = _NC_CACHE["nc"]
    in_maps = make_in_maps(inputs)
    try:
        res = run_bass_kernel_spmd(nc, in_maps, core_ids=list(range(NCORES)))
    except Exception:
        # one retry: transient NRT/device hiccups have been observed
        res = run_bass_kernel_spmd(nc, in_maps, core_ids=list(range(NCORES)))
    return assemble_out(res.results)

